# revision 1
# baseline (speedup 1.0000x reference)
"""Trainium2 Bass kernel for nn_LossConsistenciaMorfologicaCompuesta.

Composite morphological-consistency loss:
  for k in (3,5,7): Dice(pred, dilate_k(teacher)) + Dice(pred, erode_k(teacher)),
  total/3, where the structuring elements are cv2-style ellipses and Dice
  reduces over (batch, pixels).

Strategy (8 NeuronCores, data-parallel over batch B=16 -> 2 images/core):
  - Slab layout: one 1024x1024 image lives in SBUF as [128 partitions, 8+halo
    rows, 1024(+pad) cols] fp16. Vertical +-1/+-2 shifts become free-dim row
    offsets; the 2 halo rows at each slab edge are gathered with tiny
    partition-shifted SBUF->SBUF DMAs. Out-of-image halo rows use replicate
    padding, which is exact for flat morphology (a duplicated in-window pixel
    never changes a max/min).
  - Ellipse decomposition (verified exact vs the reference):
      X1   = hmax3(t)
      dil3 = max(X1, t up1, t dn1)                      (ellipse 3 = plus)
      dil5 = max(dil3 l1, dil3 r1, dil3 up1, dil3 dn1)  (ellipse 5 = diamond2)
      dil7 = max(dil5 l1/r1/up1/dn1, (t+-2,+-2) corners) (ellipse 7)
    erosion mirrored with min.
  - Per-image sums: plain sums (sum m, sum p) ride the ScalarE activation
    accumulator; product sums (sum p*m) go through PE ones-matmuls into PSUM.
  - Each core writes 22 partial sums; the host combines them into the scalar.
"""

import numpy as np

B, C_IN, H, W = 16, 1, 1024, 1024
NCORES = 8
BPC = B // NCORES      # images per core
P = 128                # SBUF partitions
R = H // P             # 8 slab rows per partition
EPS = 1e-7
PSUM_CHUNK = 512

_CACHE = {}


def build_nc(n_img=BPC, rows=R, cols=W):
    """Emit the Bass program for one core processing n_img images of
    (rows*128) x cols."""
    import concourse.bacc as bacc
    import concourse.mybir as mybir
    import concourse.tile as tile

    f32 = mybir.dt.float32
    f16 = mybir.dt.float16
    MAX = mybir.AluOpType.max
    MIN = mybir.AluOpType.min
    MULT = mybir.AluOpType.mult
    COPY = mybir.ActivationFunctionType.Copy

    Rr, C = rows, cols
    TROWS = Rr + 4          # t: 2 halo rows above + below
    MROWS = Rr + 2          # m3/m5 buffers: 1 halo row above + below
    MC = C + 4              # 2 pad cols each side
    WPLAIN = 16             # plain-sum accumulator columns
    NQ = 6                  # morph quantities: d3,d5,d7,e3,e5,e7

    nc = bacc.Bacc("TRN2", target_bir_lowering=False)
    t_dram = nc.dram_tensor("teacher", [n_img, Rr * P, C], f32, kind="ExternalInput")
    p_dram = nc.dram_tensor("pred", [n_img, Rr * P, C], f32, kind="ExternalInput")
    out_dram = nc.dram_tensor("partials", [1, 6 + WPLAIN], f32, kind="ExternalOutput")

    def halo(m):
        """Fill 1-row top/bottom halos of a morph buffer (replicate at image
        edges); pad columns ride along."""
        nc.sync.dma_start(m[1:P, 0:1, :], m[0:P - 1, MROWS - 2:MROWS - 1, :])
        nc.sync.dma_start(m[0:P - 1, MROWS - 1:MROWS, :], m[1:P, 1:2, :])
        nc.sync.dma_start(m[0:1, 0:1, :], m[0:1, 1:2, :])
        nc.sync.dma_start(m[P - 1:P, MROWS - 1:MROWS, :],
                          m[P - 1:P, MROWS - 2:MROWS - 1, :])

    with tile.TileContext(nc) as tc:
        with (
            tc.tile_pool(name="stage", bufs=2) as stage_pool,
            tc.tile_pool(name="img", bufs=1) as img_pool,
            tc.tile_pool(name="morph", bufs=1) as morph_pool,
            tc.tile_pool(name="m7", bufs=2) as m7_pool,
            tc.tile_pool(name="small", bufs=1) as small_pool,
            tc.tile_pool(name="psum", bufs=1, space="PSUM") as psum_pool,
        ):
            sums = small_pool.tile([P, WPLAIN], f32, tag="sums")
            ones16 = small_pool.tile([P, 1], f16, tag="ones16")
            ones32 = small_pool.tile([P, 1], f32, tag="ones32")
            nc.vector.memset(sums[:], 0.0)
            nc.vector.memset(ones16[:], 1.0)
            nc.vector.memset(ones32[:], 1.0)

            # long-lived image buffers (reused across images/sides)
            t = img_pool.tile([P, TROWS, C], f16, tag="t")
            p = img_pool.tile([P, Rr, C], f16, tag="p")
            h1 = morph_pool.tile([P, Rr, C], f16, tag="h1")
            mbuf = {}
            for pref, fill in (("d", -1e4), ("e", 1e4)):
                for lvl in ("3", "5"):
                    m = morph_pool.tile([P, MROWS, MC], f16, tag=pref + lvl, name=pref + lvl)
                    nc.vector.memset(m[:, :, 0:2], fill)
                    nc.vector.memset(m[:, :, MC - 2:MC], fill)
                    mbuf[pref + lvl] = m

            ps_prod = [psum_pool.tile([1, min(PSUM_CHUNK, C)], f32, tag=f"ps{q}", name=f"ps{q}")
                       for q in range(NQ)]
            n_chunks = Rr * ((C + PSUM_CHUNK - 1) // PSUM_CHUNK)
            total_mm = n_img * n_chunks
            mm_count = [0] * NQ

            def pe_sum(q, m_ap):
                """Accumulate sum over a [P, Rr, C] AP into ps_prod[q]."""
                for r in range(Rr):
                    for c0 in range(0, C, PSUM_CHUNK):
                        cw = min(PSUM_CHUNK, C - c0)
                        nc.tensor.matmul(
                            ps_prod[q][:, 0:cw],
                            ones16[:],
                            m_ap[:, r, c0:c0 + cw],
                            start=(mm_count[q] == 0),
                            stop=(mm_count[q] == total_mm - 1),
                        )
                        mm_count[q] += 1

            for img in range(n_img):
                # ---- load + cast to fp16 ----
                t_view = t_dram[img].rearrange("(p r) w -> p r w", p=P)
                p_view = p_dram[img].rearrange("(p r) w -> p r w", p=P)
                CH = 2  # slab rows per staging chunk
                for r0 in range(0, Rr, CH):
                    st = stage_pool.tile([P, CH, C], f32, tag="stage", name="stage")
                    nc.sync.dma_start(st[:], t_view[:, r0:r0 + CH, :])
                    nc.scalar.activation(t[:, 2 + r0:2 + r0 + CH, :], st[:], COPY)
                for r0 in range(0, Rr, CH):
                    st = stage_pool.tile([P, CH, C], f32, tag="stage", name="stage")
                    nc.sync.dma_start(st[:], p_view[:, r0:r0 + CH, :])
                    nc.scalar.activation(p[:, r0:r0 + CH, :], st[:], COPY)

                # ---- t halo rows (2 each side, replicate at image boundary) ----
                nc.sync.dma_start(t[1:P, 0:2, :], t[0:P - 1, Rr:Rr + 2, :])
                nc.sync.dma_start(t[0:P - 1, TROWS - 2:TROWS, :], t[1:P, 2:4, :])
                for hr in (0, 1):
                    nc.sync.dma_start(t[0:1, hr:hr + 1, :], t[0:1, 2:3, :])
                for hr in (TROWS - 2, TROWS - 1):
                    nc.sync.dma_start(t[P - 1:P, hr:hr + 1, :],
                                      t[P - 1:P, TROWS - 3:TROWS - 2, :])

                # sum(p) per partition on ACT (in-place identity copy)
                nc.scalar.activation(p[:], p[:], COPY,
                                     accum_out=sums[:, img:img + 1])

                for is_dil, base_q, pref in ((True, 0, "d"), (False, 3, "e")):
                    OP = MAX if is_dil else MIN

                    # ---- h1 = hmax3/hmin3 of t ----
                    nc.vector.tensor_tensor(h1[:, :, 1:C - 1], t[:, 2:2 + Rr, 0:C - 2],
                                            t[:, 2:2 + Rr, 2:C], op=OP)
                    nc.vector.tensor_tensor(h1[:, :, 1:C - 1], h1[:, :, 1:C - 1],
                                            t[:, 2:2 + Rr, 1:C - 1], op=OP)
                    nc.vector.tensor_tensor(h1[:, :, 0:1], t[:, 2:2 + Rr, 0:1],
                                            t[:, 2:2 + Rr, 1:2], op=OP)
                    nc.vector.tensor_tensor(h1[:, :, C - 1:C], t[:, 2:2 + Rr, C - 2:C - 1],
                                            t[:, 2:2 + Rr, C - 1:C], op=OP)

                    # ---- m3 = op(h1, t up1, t dn1) ----
                    m3 = mbuf[pref + "3"]
                    nc.vector.tensor_tensor(m3[:, 1:1 + Rr, 2:C + 2], h1[:, :, :],
                                            t[:, 3:3 + Rr, :], op=OP)
                    nc.vector.tensor_tensor(m3[:, 1:1 + Rr, 2:C + 2],
                                            m3[:, 1:1 + Rr, 2:C + 2],
                                            t[:, 1:1 + Rr, :], op=OP)
                    halo(m3)

                    # ---- m5 = op(m3 l1, r1, up1, dn1) ----
                    m5 = mbuf[pref + "5"]
                    nc.vector.tensor_tensor(m5[:, 1:1 + Rr, 2:C + 2],
                                            m3[:, 1:1 + Rr, 1:C + 1],
                                            m3[:, 1:1 + Rr, 3:C + 3], op=OP)
                    nc.vector.tensor_tensor(m5[:, 1:1 + Rr, 2:C + 2],
                                            m5[:, 1:1 + Rr, 2:C + 2],
                                            m3[:, 2:2 + Rr, 2:C + 2], op=OP)
                    nc.vector.tensor_tensor(m5[:, 1:1 + Rr, 2:C + 2],
                                            m5[:, 1:1 + Rr, 2:C + 2],
                                            m3[:, 0:Rr, 2:C + 2], op=OP)
                    halo(m5)

                    # ---- m7 = op(m5 l1/r1/up1/dn1, t corner terms) ----
                    m7 = m7_pool.tile([P, Rr, C], f16, tag="m7", name="m7")
                    nc.vector.tensor_tensor(m7[:], m5[:, 1:1 + Rr, 1:C + 1],
                                            m5[:, 1:1 + Rr, 3:C + 3], op=OP)
                    nc.vector.tensor_tensor(m7[:], m7[:],
                                            m5[:, 2:2 + Rr, 2:C + 2], op=OP)
                    nc.vector.tensor_tensor(m7[:], m7[:],
                                            m5[:, 0:Rr, 2:C + 2], op=OP)
                    # corners: (t up2 / dn2) shifted +-2 cols, col-restricted
                    nc.vector.tensor_tensor(m7[:, :, 2:C], m7[:, :, 2:C],
                                            t[:, 4:4 + Rr, 0:C - 2], op=OP)
                    nc.vector.tensor_tensor(m7[:, :, 0:C - 2], m7[:, :, 0:C - 2],
                                            t[:, 4:4 + Rr, 2:C], op=OP)
                    nc.vector.tensor_tensor(m7[:, :, 2:C], m7[:, :, 2:C],
                                            t[:, 0:Rr, 0:C - 2], op=OP)
                    nc.vector.tensor_tensor(m7[:, :, 0:C - 2], m7[:, :, 0:C - 2],
                                            t[:, 0:Rr, 2:C], op=OP)

                    # ---- sums + products ----
                    col = 2 + img * 6
                    m3i = m3[:, 1:1 + Rr, 2:C + 2]
                    m5i = m5[:, 1:1 + Rr, 2:C + 2]
                    for qi, m_ap in ((0, m3i), (1, m5i), (2, m7[:, :, :])):
                        q = base_q + qi
                        nc.scalar.activation(m_ap, m_ap, COPY,
                                             accum_out=sums[:, col + q:col + q + 1])
                        nc.vector.tensor_tensor(m_ap, m_ap, p[:], op=MULT)
                        pe_sum(q, m_ap)

            # ---- epilogue ----
            CW = min(PSUM_CHUNK, C)
            prodsb = small_pool.tile([1, NQ * CW], f32, tag="prodsb")
            outsb = small_pool.tile([1, 6 + WPLAIN], f32, tag="outsb")
            for q in range(NQ):
                nc.scalar.activation(prodsb[:, q * CW:(q + 1) * CW],
                                     ps_prod[q][:], COPY)
            nc.vector.tensor_reduce(
                outsb[:, 0:NQ],
                prodsb[:, :].rearrange("p (q k) -> p q k", k=CW),
                axis=mybir.AxisListType.X,
                op=mybir.AluOpType.add,
            )
            ps_plain = psum_pool.tile([1, WPLAIN], f32, tag="psplain")
            nc.tensor.matmul(ps_plain[:], ones32[:], sums[:], start=True, stop=True)
            nc.scalar.activation(outsb[:, NQ:NQ + WPLAIN], ps_plain[:], COPY)
            nc.sync.dma_start(out_dram[:], outsb[:])

    nc.compile()
    return nc


def combine_partials(partials, n_img=BPC):
    """partials: [ncores, 22] float32 -> scalar loss (mirrors reference math)."""
    partials = np.asarray(partials, dtype=np.float64)
    prod_sums = partials[:, 0:6].sum(axis=0)            # sum p*m per quantity
    plain = partials[:, 6:]                             # [ncores, 16]
    p_sum = plain[:, 0:n_img].sum()
    m_sums = np.zeros(6)
    for img in range(n_img):
        m_sums += plain[:, 2 + img * 6:2 + img * 6 + 6].sum(axis=0)
    total = 0.0
    for q in range(6):
        card = p_sum + m_sums[q]
        score = 2.0 * prod_sums[q] / max(card, EPS)
        loss = (1.0 - score) * (1.0 if m_sums[q] > 0 else 0.0)
        total += loss
    return np.float32(total / 3.0)


def kernel(pred_student_prob, teacher_prob):
    from concourse.bass_utils import run_bass_kernel_spmd

    key = (BPC, R, W)
    if key not in _CACHE:
        _CACHE[key] = build_nc(BPC, R, W)
    nc = _CACHE[key]

    pred = np.ascontiguousarray(pred_student_prob.reshape(B, H, W), dtype=np.float32)
    teach = np.ascontiguousarray(teacher_prob.reshape(B, H, W), dtype=np.float32)
    in_maps = []
    for c in range(NCORES):
        sl = slice(c * BPC, (c + 1) * BPC)
        in_maps.append({
            "teacher": np.ascontiguousarray(teach[sl]),
            "pred": np.ascontiguousarray(pred[sl]),
        })
    res = run_bass_kernel_spmd(nc, in_maps, core_ids=list(range(NCORES)))
    partials = np.stack([res.results[c]["partials"][0] for c in range(NCORES)])
    return combine_partials(partials)



# revision 3
# speedup vs baseline: 1.2520x; 1.2520x over previous
"""Trainium2 Bass kernel for nn_LossConsistenciaMorfologicaCompuesta (v2).

Composite morphological-consistency loss:
  for k in (3,5,7): Dice(pred, dilate_k(teacher)) + Dice(pred, erode_k(teacher)),
  total/3, cv2-style ellipse structuring elements, Dice reduced over
  (batch, pixels).

Strategy (8 NeuronCores, data-parallel over batch B=16 -> 2 images/core):
  - Slab layout: image = [128 partitions, 8 rows, 1024 cols] fp16; vertical
    shifts are free-dim row offsets. Slab-crossing halo rows are built by the
    PE with shifted-identity matmuls (partition shift) + ACT PSUM->SBUF
    copies, with replicate edges (exact for flat morphology).
  - Ellipse decomposition (verified exact vs reference):
      h1 = hmax3(t); m3 = max(h1, t up1, t dn1)
      m5 = max(m3 l1, r1, up1, dn1)
      m7 = max(m5 l1, r1, up1, dn1) merged with corner term
           C = max(t up2, dn2) shifted +-2 cols
    erosion mirrored with min. 16 full-size DVE passes per (image, side).
  - Sums: sum(p) rides the input-cast ACT accumulator; sum(m) via ACT copy
    into a dump tile with accum_out; sum(p*m) via DVE mult into rotating
    product tiles + PE ones-matmul PSUM chains.
  - Each core writes 30 partial sums; the host combines them into the loss.
"""

import numpy as np

B, C_IN, H, W = 16, 1, 1024, 1024
NCORES = 8
BPC = B // NCORES      # images per core
P = 128                # SBUF partitions
R = H // P             # 8 slab rows per partition
EPS = 1e-7
CW = 512               # PSUM chunk width

NQ = 6                 # d3,d5,d7,e3,e5,e7
NPLAIN = 32            # plain-sum columns: 8 p-chunks + 24 m-sum halves
NOUT = NQ + NPLAIN

_CACHE = {}


def _shift_identity(up: bool) -> np.ndarray:
    """W[p, k] = 1 iff p == k-1 (up) / p == k+1 (down); edge row zeroed
    (edge halo handled by an explicit replicate copy)."""
    w = np.zeros((P, P), np.float16)
    if up:
        for k in range(1, P):
            w[k - 1, k] = 1.0
    else:
        for k in range(P - 1):
            w[k + 1, k] = 1.0
    return w


def build_nc(n_img=BPC, rows=R, cols=W):
    import concourse.bacc as bacc
    import concourse.mybir as mybir
    import concourse.tile as tile

    f32 = mybir.dt.float32
    f16 = mybir.dt.float16
    MAX = mybir.AluOpType.max
    MIN = mybir.AluOpType.min
    MULT = mybir.AluOpType.mult
    COPY = mybir.ActivationFunctionType.Copy

    Rr, C = rows, cols
    TROWS = Rr + 4          # t: rows idx 0,1 = image rows -2,-1; 2..9 = 0..7;
                            # 10,11 = rows 8,9
    MROWS = Rr + 2          # m3/m5: row idx 0 = halo -1, 1..8 interior, 9 = halo
    MC = C + 2              # 1 pad col each side

    nc = bacc.Bacc("TRN2", target_bir_lowering=False)
    t_dram = nc.dram_tensor("teacher", [n_img, Rr * P, C], f32, kind="ExternalInput")
    p_dram = nc.dram_tensor("pred", [n_img, Rr * P, C], f32, kind="ExternalInput")
    wup_dram = nc.dram_tensor("wup", [P, P], f16, kind="ExternalInput")
    wdn_dram = nc.dram_tensor("wdn", [P, P], f16, kind="ExternalInput")
    out_dram = nc.dram_tensor("partials", [1, NOUT], f32, kind="ExternalOutput")

    with tile.TileContext(nc) as tc:
        with (
            tc.tile_pool(name="stage", bufs=3) as stage_pool,
            tc.tile_pool(name="img", bufs=1) as img_pool,
            tc.tile_pool(name="morph", bufs=1) as morph_pool,
            tc.tile_pool(name="prod", bufs=2) as prod_pool,
            tc.tile_pool(name="small", bufs=1) as small_pool,
            tc.tile_pool(name="pprod", bufs=1, space="PSUM") as pprod_pool,
            tc.tile_pool(name="phalo", bufs=2, space="PSUM") as phalo_pool,
        ):
            sums = small_pool.tile([P, NPLAIN], f32, tag="sums")
            ones16 = small_pool.tile([P, 1], f16, tag="ones16")
            ones32 = small_pool.tile([P, 1], f32, tag="ones32")
            outsb = small_pool.tile([1, NOUT], f32, tag="outsb")
            wup = small_pool.tile([P, P], f16, tag="wup")
            wdn = small_pool.tile([P, P], f16, tag="wdn")
            nc.vector.memset(sums[:], 0.0)
            nc.vector.memset(ones16[:], 1.0)
            nc.vector.memset(ones32[:], 1.0)


            t = img_pool.tile([P, TROWS, C], f16, tag="t")
            p = img_pool.tile([P, Rr, C], f16, tag="p")
            dump = img_pool.tile([P, Rr // 2, C], f16, tag="dump")
            m7t = img_pool.tile([P, Rr, C], f16, tag="m7t")
            mbuf = {}
            for pref, fill in (("d", -1e4), ("e", 1e4)):
                for lvl in ("3", "5"):
                    m = morph_pool.tile([P, MROWS, MC], f16, tag=pref + lvl,
                                        name=pref + lvl)
                    nc.vector.memset(m[:, :, 0:1], fill)
                    nc.vector.memset(m[:, :, MC - 1:MC], fill)
                    mbuf[pref + lvl] = m

            ps_prod = [pprod_pool.tile([1, CW], f32, tag=f"ps{q}", name=f"ps{q}")
                       for q in range(NQ)]
            nch = (C + CW - 1) // CW
            total_mm = n_img * Rr * nch
            mm_count = [0] * NQ

            def pe_sum_rows(q, m_ap, ra, rb):
                """Accumulate sum over rows [ra, rb) of a [P, Rr, C] AP into
                ps_prod[q] (fp16 ones-matmuls)."""
                for r in range(ra, rb):
                    for c0 in range(0, C, CW):
                        nc.tensor.matmul(
                            ps_prod[q][:, 0:CW],
                            ones16[:],
                            m_ap[:, r, c0:c0 + CW],
                            start=(mm_count[q] == 0),
                            stop=(mm_count[q] == total_mm - 1),
                        )
                        mm_count[q] += 1

            def pe_sum(q, m_ap):
                pe_sum_rows(q, m_ap, 0, Rr)

            def pe_halo_row(dst_row_ap, w_ap, src_row_ap, tag):
                """dst_row[1:P or 0:P-1] = partition-shifted src_row via PE;
                edge partition left for a separate replicate copy."""
                for ci, c0 in enumerate(range(0, C, CW)):
                    ph = phalo_pool.tile([P, CW], f32, tag="phalo",
                                         name=f"ph_{tag}_{ci}")
                    nc.tensor.matmul(ph[:], w_ap[:], src_row_ap[:, c0:c0 + CW],
                                     start=True, stop=True)
                    nc.scalar.activation(dst_row_ap[:, c0:c0 + CW], ph[:, :],
                                         COPY)

            def m_halo(m, side):
                """Fill m's halo rows 0 and MROWS-1 (interior cols) via PE;
                replicate at image top/bottom edges."""
                pe_halo_row(m[:, 0, 1:C + 1], wup, m[:, MROWS - 2, 1:C + 1],
                            f"{side}u")
                pe_halo_row(m[:, MROWS - 1, 1:C + 1], wdn, m[:, 1, 1:C + 1],
                            f"{side}d")
                nc.scalar.activation(m[0:1, 0, 1:C + 1], m[0:1, 1, 1:C + 1], COPY)
                nc.sync.dma_start(m[P - 1:P, MROWS - 1:MROWS, 1:C + 1],
                                  m[P - 1:P, MROWS - 2:MROWS - 1, 1:C + 1])

            def act_msum(m_ap, slot):
                """sum(m) on ACT in two half-dumps; accum cols 8+2*slot, +1."""
                half = Rr // 2
                for hi in range(2):
                    nc.scalar.activation(dump[:], m_ap[:, hi * half:(hi + 1) * half, :],
                                         COPY,
                                         accum_out=sums[:, 8 + 2 * slot + hi:
                                                        9 + 2 * slot + hi])

            pdump = small_pool.tile([1, CW], f32, tag="pdump")
            pdumped = set()

            def pdump_q(q):
                pdumped.add(q)
                nc.scalar.activation(pdump[:], ps_prod[q][:], COPY,
                                     accum_out=outsb[:, q:q + 1])

            for img in range(n_img):
                t_view = t_dram[img].rearrange("(p r) w -> p r w", p=P)
                p_view = p_dram[img].rearrange("(p r) w -> p r w", p=P)
                CH = 2
                # ---- teacher load + cast (Pool tensor_copy; img 0 uses
                # 1-row chunks alternating with ACT so the first h1 can
                # start sooner) ----
                TCH = 1 if img == 0 else CH
                for ci, r0 in enumerate(range(0, Rr, TCH)):
                    st = stage_pool.tile([P, CH, C], f32, tag="stage", name="st")
                    nc.sync.dma_start(st[:, 0:TCH, :], t_view[:, r0:r0 + TCH, :])
                    dst = t[:, 2 + r0:2 + r0 + TCH, :]
                    if img == 0 and ci % 2 == 0:
                        nc.scalar.activation(dst, st[:, 0:TCH, :], COPY)
                    else:
                        nc.gpsimd.tensor_copy(dst, st[:, 0:TCH, :])
                if img == 0:
                    nc.sync.dma_start(wup[:], wup_dram[:])
                    nc.sync.dma_start(wdn[:], wdn_dram[:])
                # ---- t halo rows via PE shift + replicate edges ----
                pe_halo_row(t[:, 1, :], wup, t[:, 9, :], "t1")   # row -1
                pe_halo_row(t[:, 0, :], wup, t[:, 8, :], "t0")   # row -2
                pe_halo_row(t[:, 10, :], wdn, t[:, 2, :], "t10")  # row 8
                pe_halo_row(t[:, 11, :], wdn, t[:, 3, :], "t11")  # row 9
                for hr in (0, 1):
                    nc.scalar.activation(t[0:1, hr, :], t[0:1, 2, :], COPY)
                for hr in (10, 11):
                    nc.sync.dma_start(t[P - 1:P, hr:hr + 1, :],
                                      t[P - 1:P, 9:10, :])
                # ---- pred load + cast (accum -> p plain sums) ----
                for ci, r0 in enumerate(range(0, Rr, CH)):
                    st = stage_pool.tile([P, CH, C], f32, tag="stage", name="st")
                    nc.sync.dma_start(st[:], p_view[:, r0:r0 + CH, :])
                    nc.scalar.activation(p[:, r0:r0 + CH, :], st[:], COPY,
                                         accum_out=sums[:, img * 4 + ci:
                                                        img * 4 + ci + 1])

                sides = (("d", MAX, 0), ("e", MIN, 3))

                # h1 into m5 interior (alias; dead once m5 is written).
                # For img 0 split into row halves so work starts after the
                # first two cast chunks.
                h1 = {pref: mbuf[pref + "5"][:, 1:1 + Rr, 1:C + 1]
                      for pref, _, _ in sides}
                ti = t[:, 2:2 + Rr, :]     # image rows 0..7
                row_chunks = ((0, 1), (1, 2), (2, 4), (4, 8)) if img == 0 else ((0, 8),)
                for ra, rb in row_chunks:
                    for pref, OP, _ in sides:
                        h = h1[pref][:, ra:rb, :]
                        tc_ = ti[:, ra:rb, :]
                        nc.vector.tensor_tensor(h[:, :, 1:C - 1], tc_[:, :, 0:C - 2],
                                                tc_[:, :, 2:C], op=OP)
                        nc.vector.tensor_tensor(h[:, :, 1:C - 1], h[:, :, 1:C - 1],
                                                tc_[:, :, 1:C - 1], op=OP)
                        nc.vector.tensor_tensor(h[:, :, 0:1], tc_[:, :, 0:1],
                                                tc_[:, :, 1:2], op=OP)
                        nc.vector.tensor_tensor(h[:, :, C - 1:C],
                                                tc_[:, :, C - 2:C - 1],
                                                tc_[:, :, C - 1:C], op=OP)

                # m3 = op(h1, t up1, t dn1)
                for pref, OP, _ in sides:
                    m3 = mbuf[pref + "3"]
                    nc.vector.tensor_tensor(m3[:, 1:1 + Rr, 1:C + 1], h1[pref],
                                            t[:, 1:1 + Rr, :], op=OP)
                    nc.vector.tensor_tensor(m3[:, 1:1 + Rr, 1:C + 1],
                                            m3[:, 1:1 + Rr, 1:C + 1],
                                            t[:, 3:3 + Rr, :], op=OP)
                for pref, OP, _ in sides:
                    m_halo(mbuf[pref + "3"], pref + "3")
                for pref, OP, _ in sides:
                    act_msum(mbuf[pref + "3"][:, 1:1 + Rr, 1:C + 1],
                             img * 6 + (0 if pref == "d" else 3))

                # products for m3 while halos fly
                for pref, OP, base_q in sides:
                    pr = prod_pool.tile([P, Rr, C], f16, tag="prod",
                                        name=f"pr3{pref}")
                    nc.vector.tensor_tensor(pr[:],
                                            mbuf[pref + "3"][:, 1:1 + Rr, 1:C + 1],
                                            p[:], op=MULT)
                    pe_sum(base_q + 0, pr[:, :, :])

                # m5 = op(m3 l1, r1, up1, dn1)
                for pref, OP, _ in sides:
                    m3 = mbuf[pref + "3"]
                    m5 = mbuf[pref + "5"]
                    nc.vector.tensor_tensor(m5[:, 1:1 + Rr, 1:C + 1],
                                            m3[:, 1:1 + Rr, 0:C],
                                            m3[:, 1:1 + Rr, 2:C + 2], op=OP)
                    nc.vector.tensor_tensor(m5[:, 1:1 + Rr, 1:C + 1],
                                            m5[:, 1:1 + Rr, 1:C + 1],
                                            m3[:, 0:Rr, 1:C + 1], op=OP)
                    nc.vector.tensor_tensor(m5[:, 1:1 + Rr, 1:C + 1],
                                            m5[:, 1:1 + Rr, 1:C + 1],
                                            m3[:, 2:2 + Rr, 1:C + 1], op=OP)
                for pref, OP, _ in sides:
                    m_halo(mbuf[pref + "5"], pref + "5")
                for pref, OP, _ in sides:
                    act_msum(mbuf[pref + "5"][:, 1:1 + Rr, 1:C + 1],
                             img * 6 + 1 + (0 if pref == "d" else 3))


                # products for m5
                for pref, OP, base_q in sides:
                    pr = prod_pool.tile([P, Rr, C], f16, tag="prod",
                                        name=f"pr5{pref}")
                    nc.vector.tensor_tensor(pr[:],
                                            mbuf[pref + "5"][:, 1:1 + Rr, 1:C + 1],
                                            p[:], op=MULT)
                    pe_sum(base_q + 1, pr[:, :, :])

                # corner terms M = op(t up2, t dn2) into prod-pool tiles
                Ms = {}
                for pref, OP, _ in sides:
                    M = prod_pool.tile([P, Rr, C], f16, tag="prod",
                                       name=f"M{pref}")
                    nc.vector.tensor_tensor(M[:], t[:, 0:Rr, :],
                                            t[:, 4:4 + Rr, :], op=OP)
                    Ms[pref] = M

                # m7 (uses m7t, one side at a time: d fully then e)
                for pref, OP, base_q in sides:
                    m5 = mbuf[pref + "5"]
                    M = Ms[pref]
                    # d-side uses m7t; e-side reuses m3_e's dead interior so
                    # the two sides' buffers are independent (no WAR with the
                    # d-side's ACT sum)
                    m7o = (m7t if pref == "d"
                           else mbuf["e3"][:, 1:1 + Rr, 1:C + 1])
                    nc.vector.tensor_tensor(m7o[:], m5[:, 1:1 + Rr, 0:C],
                                            m5[:, 1:1 + Rr, 2:C + 2], op=OP)
                    nc.vector.tensor_tensor(m7o[:], m7o[:],
                                            m5[:, 0:Rr, 1:C + 1], op=OP)
                    nc.vector.tensor_tensor(m7o[:], m7o[:],
                                            m5[:, 2:2 + Rr, 1:C + 1], op=OP)
                    # corner merges, col-restricted
                    nc.vector.tensor_tensor(m7o[:, :, 2:C], m7o[:, :, 2:C],
                                            M[:, :, 0:C - 2], op=OP)
                    nc.vector.tensor_tensor(m7o[:, :, 0:C - 2], m7o[:, :, 0:C - 2],
                                            M[:, :, 2:C], op=OP)
                    act_msum(m7o[:], img * 6 + 2 + (0 if pref == "d" else 3))
                    # split the m7 product into row pairs so its PE sum chain
                    # overlaps the remaining DVE work (shortens the tail)
                    pr = prod_pool.tile([P, Rr, C], f16, tag="prod",
                                        name=f"pr7{pref}")
                    last = (img == n_img - 1 and pref == "e")
                    chunks = ((0, 2), (2, 4), (4, 6), (6, 7), (7, 8)) if last \
                        else ((0, 2), (2, 4), (4, 6), (6, 8))
                    for ra, rb in chunks:
                        nc.vector.tensor_tensor(pr[:, ra:rb, :],
                                                m7o[:, ra:rb, :],
                                                p[:, ra:rb, :], op=MULT)
                        pe_sum_rows(base_q + 2, pr[:, :, :], ra, rb)

            # ---- epilogue ----
            for q in (0, 1, 2):
                nc.vector.tensor_reduce(outsb[:, q:q + 1], ps_prod[q][:],
                                        axis=mybir.AxisListType.X,
                                        op=mybir.AluOpType.add)
            for q in (3, 4, 5):
                pdump_q(q)
            ps_plain = phalo_pool.tile([1, NPLAIN], f32, tag="phalo",
                                       name="ps_plain")
            nc.tensor.matmul(ps_plain[:], ones32[:], sums[:], start=True,
                             stop=True)
            nc.scalar.activation(outsb[:, NQ:NQ + NPLAIN], ps_plain[:], COPY)
            nc.sync.dma_start(out_dram[:], outsb[:])

    nc.compile()
    return nc


def combine_partials(partials, n_img=BPC):
    """partials: [ncores, NOUT] float32 -> scalar loss."""
    partials = np.asarray(partials, dtype=np.float64)
    prod_sums = partials[:, 0:NQ].sum(axis=0)
    plain = partials[:, NQ:]
    p_sum = plain[:, 0:8].sum()
    m_sums = np.zeros(NQ)
    for img in range(n_img):
        for q in range(NQ):
            slot = img * 6 + q
            m_sums[q] += plain[:, 8 + 2 * slot:10 + 2 * slot].sum()
    total = 0.0
    for q in range(NQ):
        card = p_sum + m_sums[q]
        score = 2.0 * prod_sums[q] / max(card, EPS)
        total += (1.0 - score) * (1.0 if m_sums[q] > 0 else 0.0)
    return np.float32(total / 3.0)


def kernel(pred_student_prob, teacher_prob):
    from concourse.bass_utils import run_bass_kernel_spmd

    key = (BPC, R, W)
    if key not in _CACHE:
        _CACHE[key] = build_nc(BPC, R, W)
    nc = _CACHE[key]

    pred = np.ascontiguousarray(pred_student_prob.reshape(B, H, W), dtype=np.float32)
    teach = np.ascontiguousarray(teacher_prob.reshape(B, H, W), dtype=np.float32)
    wup = _shift_identity(True)
    wdn = _shift_identity(False)
    in_maps = []
    for c in range(NCORES):
        sl = slice(c * BPC, (c + 1) * BPC)
        in_maps.append({
            "teacher": np.ascontiguousarray(teach[sl]),
            "pred": np.ascontiguousarray(pred[sl]),
            "wup": wup,
            "wdn": wdn,
        })
    res = run_bass_kernel_spmd(nc, in_maps, core_ids=list(range(NCORES)))
    partials = np.stack([res.results[c]["partials"][0] for c in range(NCORES)])
    return combine_partials(partials)


# revision 4
# speedup vs baseline: 1.2715x; 1.0155x over previous
"""Trainium2 Bass kernel for nn_LossConsistenciaMorfologicaCompuesta (v2).

Composite morphological-consistency loss:
  for k in (3,5,7): Dice(pred, dilate_k(teacher)) + Dice(pred, erode_k(teacher)),
  total/3, cv2-style ellipse structuring elements, Dice reduced over
  (batch, pixels).

Strategy (8 NeuronCores, data-parallel over batch B=16 -> 2 images/core):
  - Slab layout: image = [128 partitions, 8 rows, 1024 cols] fp16; vertical
    shifts are free-dim row offsets. Slab-crossing halo rows are built by the
    PE with shifted-identity matmuls (partition shift) + ACT PSUM->SBUF
    copies, with replicate edges (exact for flat morphology).
  - Ellipse decomposition (verified exact vs reference):
      h1 = hmax3(t); m3 = max(h1, t up1, t dn1)
      m5 = max(m3 l1, r1, up1, dn1)
      m7 = max(m5 l1, r1, up1, dn1) merged with corner term
           C = max(t up2, dn2) shifted +-2 cols
    erosion mirrored with min. 16 full-size DVE passes per (image, side).
  - Sums: sum(p) rides the input-cast ACT accumulator; sum(m) via ACT copy
    into a dump tile with accum_out; sum(p*m) via DVE mult into rotating
    product tiles + PE ones-matmul PSUM chains.
  - Each core writes 30 partial sums; the host combines them into the loss.
"""

import numpy as np

B, C_IN, H, W = 16, 1, 1024, 1024
NCORES = 8
BPC = B // NCORES      # images per core
P = 128                # SBUF partitions
R = H // P             # 8 slab rows per partition
EPS = 1e-7
CW = 512               # PSUM chunk width

NQ = 6                 # d3,d5,d7,e3,e5,e7
NPLAIN = 32            # plain-sum columns: 8 p-chunks + 24 m-sum halves
NOUT = NQ + NPLAIN

_CACHE = {}


def _shift_identity(up: bool) -> np.ndarray:
    """W[p, k] = 1 iff p == k-1 (up) / p == k+1 (down); edge row zeroed
    (edge halo handled by an explicit replicate copy)."""
    w = np.zeros((P, P), np.float16)
    if up:
        for k in range(1, P):
            w[k - 1, k] = 1.0
    else:
        for k in range(P - 1):
            w[k + 1, k] = 1.0
    return w


def build_nc(n_img=BPC, rows=R, cols=W):
    import concourse.bacc as bacc
    import concourse.mybir as mybir
    import concourse.tile as tile

    f32 = mybir.dt.float32
    f16 = mybir.dt.float16
    MAX = mybir.AluOpType.max
    MIN = mybir.AluOpType.min
    MULT = mybir.AluOpType.mult
    COPY = mybir.ActivationFunctionType.Copy

    Rr, C = rows, cols
    TROWS = Rr + 4          # t: rows idx 0,1 = image rows -2,-1; 2..9 = 0..7;
                            # 10,11 = rows 8,9
    MROWS = Rr + 2          # m3/m5: row idx 0 = halo -1, 1..8 interior, 9 = halo
    MC = C + 2              # 1 pad col each side

    nc = bacc.Bacc("TRN2", target_bir_lowering=False)
    t_dram = nc.dram_tensor("teacher", [n_img, Rr * P, C], f16, kind="ExternalInput")
    p_dram = nc.dram_tensor("pred", [n_img, Rr * P, C], f16, kind="ExternalInput")
    wup_dram = nc.dram_tensor("wup", [P, P], f16, kind="ExternalInput")
    wdn_dram = nc.dram_tensor("wdn", [P, P], f16, kind="ExternalInput")
    out_dram = nc.dram_tensor("partials", [1, NOUT], f32, kind="ExternalOutput")

    with tile.TileContext(nc) as tc:
        with (
            tc.tile_pool(name="img", bufs=1) as img_pool,
            tc.tile_pool(name="morph", bufs=1) as morph_pool,
            tc.tile_pool(name="prod", bufs=2) as prod_pool,
            tc.tile_pool(name="small", bufs=1) as small_pool,
            tc.tile_pool(name="pprod", bufs=1, space="PSUM") as pprod_pool,
            tc.tile_pool(name="phalo", bufs=2, space="PSUM") as phalo_pool,
        ):
            sums = small_pool.tile([P, NPLAIN], f32, tag="sums")
            ones16 = small_pool.tile([P, 1], f16, tag="ones16")
            ones32 = small_pool.tile([P, 1], f32, tag="ones32")
            outsb = small_pool.tile([1, NOUT], f32, tag="outsb")
            wup = small_pool.tile([P, P], f16, tag="wup")
            wdn = small_pool.tile([P, P], f16, tag="wdn")
            nc.vector.memset(sums[:], 0.0)
            nc.vector.memset(ones16[:], 1.0)
            nc.vector.memset(ones32[:], 1.0)


            t = img_pool.tile([P, TROWS, C], f16, tag="t")
            p = img_pool.tile([P, Rr, C], f16, tag="p")
            dump = img_pool.tile([P, Rr // 2, C], f16, tag="dump")
            m7t = img_pool.tile([P, Rr, C], f16, tag="m7t")
            mbuf = {}
            for pref, fill in (("d", -1e4), ("e", 1e4)):
                for lvl in ("3", "5"):
                    m = morph_pool.tile([P, MROWS, MC], f16, tag=pref + lvl,
                                        name=pref + lvl)
                    nc.vector.memset(m[:, :, 0:1], fill)
                    nc.vector.memset(m[:, :, MC - 1:MC], fill)
                    mbuf[pref + lvl] = m

            ps_prod = [pprod_pool.tile([1, CW], f32, tag=f"ps{q}", name=f"ps{q}")
                       for q in range(NQ)]
            nch = (C + CW - 1) // CW
            total_mm = n_img * Rr * nch
            mm_count = [0] * NQ

            def pe_sum_rows(q, m_ap, ra, rb):
                """Accumulate sum over rows [ra, rb) of a [P, Rr, C] AP into
                ps_prod[q] (fp16 ones-matmuls)."""
                for r in range(ra, rb):
                    for c0 in range(0, C, CW):
                        nc.tensor.matmul(
                            ps_prod[q][:, 0:CW],
                            ones16[:],
                            m_ap[:, r, c0:c0 + CW],
                            start=(mm_count[q] == 0),
                            stop=(mm_count[q] == total_mm - 1),
                        )
                        mm_count[q] += 1

            def pe_sum(q, m_ap):
                pe_sum_rows(q, m_ap, 0, Rr)

            def pe_halo_row(dst_row_ap, w_ap, src_row_ap, tag):
                """dst_row[1:P or 0:P-1] = partition-shifted src_row via PE;
                edge partition left for a separate replicate copy."""
                for ci, c0 in enumerate(range(0, C, CW)):
                    ph = phalo_pool.tile([P, CW], f32, tag="phalo",
                                         name=f"ph_{tag}_{ci}")
                    nc.tensor.matmul(ph[:], w_ap[:], src_row_ap[:, c0:c0 + CW],
                                     start=True, stop=True)
                    nc.scalar.activation(dst_row_ap[:, c0:c0 + CW], ph[:, :],
                                         COPY)

            def m_halo(m, side):
                """Fill m's halo rows 0 and MROWS-1 (interior cols) via PE;
                replicate at image top/bottom edges."""
                pe_halo_row(m[:, 0, 1:C + 1], wup, m[:, MROWS - 2, 1:C + 1],
                            f"{side}u")
                pe_halo_row(m[:, MROWS - 1, 1:C + 1], wdn, m[:, 1, 1:C + 1],
                            f"{side}d")
                nc.scalar.activation(m[0:1, 0, 1:C + 1], m[0:1, 1, 1:C + 1], COPY)
                nc.sync.dma_start(m[P - 1:P, MROWS - 1:MROWS, 1:C + 1],
                                  m[P - 1:P, MROWS - 2:MROWS - 1, 1:C + 1])

            def act_msum(m_ap, slot):
                """sum(m) on ACT in two half-dumps; accum cols 8+2*slot, +1."""
                half = Rr // 2
                for hi in range(2):
                    nc.scalar.activation(dump[:], m_ap[:, hi * half:(hi + 1) * half, :],
                                         COPY,
                                         accum_out=sums[:, 8 + 2 * slot + hi:
                                                        9 + 2 * slot + hi])

            pdump = small_pool.tile([1, CW], f32, tag="pdump")
            pdumped = set()

            def pdump_q(q):
                pdumped.add(q)
                nc.scalar.activation(pdump[:], ps_prod[q][:], COPY,
                                     accum_out=outsb[:, q:q + 1])

            for img in range(n_img):
                t_view = t_dram[img].rearrange("(p r) w -> p r w", p=P)
                p_view = p_dram[img].rearrange("(p r) w -> p r w", p=P)
                CH = 2
                # ---- teacher load: direct fp16 DMA (host pre-casts).
                # img 0 uses small leading chunks so h1 starts early. ----
                tchunks = ((0, 1), (1, 2), (2, 4), (4, 8)) if img == 0 \
                    else ((0, 4), (4, 8))
                for r0, r1 in tchunks:
                    nc.sync.dma_start(t[:, 2 + r0:2 + r1, :],
                                      t_view[:, r0:r1, :])
                if img == 0:
                    nc.sync.dma_start(wup[:], wup_dram[:])
                    nc.sync.dma_start(wdn[:], wdn_dram[:])
                # ---- t halo rows via PE shift + replicate edges ----
                pe_halo_row(t[:, 1, :], wup, t[:, 9, :], "t1")   # row -1
                pe_halo_row(t[:, 0, :], wup, t[:, 8, :], "t0")   # row -2
                pe_halo_row(t[:, 10, :], wdn, t[:, 2, :], "t10")  # row 8
                pe_halo_row(t[:, 11, :], wdn, t[:, 3, :], "t11")  # row 9
                for hr in (0, 1):
                    nc.scalar.activation(t[0:1, hr, :], t[0:1, 2, :], COPY)
                for hr in (10, 11):
                    nc.sync.dma_start(t[P - 1:P, hr:hr + 1, :],
                                      t[P - 1:P, 9:10, :])
                # ---- pred load: direct fp16 DMA; sum(p) via ACT dumps ----
                for r0, r1 in ((0, 4), (4, 8)):
                    nc.sync.dma_start(p[:, r0:r1, :], p_view[:, r0:r1, :])
                half = Rr // 2
                for hi in range(2):
                    nc.scalar.activation(dump[:], p[:, hi * half:(hi + 1) * half, :],
                                         COPY,
                                         accum_out=sums[:, img * 2 + hi:
                                                        img * 2 + hi + 1])

                sides = (("d", MAX, 0), ("e", MIN, 3))

                # h1 into m5 interior (alias; dead once m5 is written).
                # For img 0 split into row halves so work starts after the
                # first two cast chunks.
                h1 = {pref: mbuf[pref + "5"][:, 1:1 + Rr, 1:C + 1]
                      for pref, _, _ in sides}
                ti = t[:, 2:2 + Rr, :]     # image rows 0..7
                row_chunks = ((0, 1), (1, 2), (2, 4), (4, 8)) if img == 0 else ((0, 8),)
                for ra, rb in row_chunks:
                    for pref, OP, _ in sides:
                        h = h1[pref][:, ra:rb, :]
                        tc_ = ti[:, ra:rb, :]
                        nc.vector.tensor_tensor(h[:, :, 1:C - 1], tc_[:, :, 0:C - 2],
                                                tc_[:, :, 2:C], op=OP)
                        nc.vector.tensor_tensor(h[:, :, 1:C - 1], h[:, :, 1:C - 1],
                                                tc_[:, :, 1:C - 1], op=OP)
                        nc.vector.tensor_tensor(h[:, :, 0:1], tc_[:, :, 0:1],
                                                tc_[:, :, 1:2], op=OP)
                        nc.vector.tensor_tensor(h[:, :, C - 1:C],
                                                tc_[:, :, C - 2:C - 1],
                                                tc_[:, :, C - 1:C], op=OP)

                # m3 = op(h1, t up1, t dn1)
                for pref, OP, _ in sides:
                    m3 = mbuf[pref + "3"]
                    nc.vector.tensor_tensor(m3[:, 1:1 + Rr, 1:C + 1], h1[pref],
                                            t[:, 1:1 + Rr, :], op=OP)
                    nc.vector.tensor_tensor(m3[:, 1:1 + Rr, 1:C + 1],
                                            m3[:, 1:1 + Rr, 1:C + 1],
                                            t[:, 3:3 + Rr, :], op=OP)
                for pref, OP, _ in sides:
                    m_halo(mbuf[pref + "3"], pref + "3")
                for pref, OP, _ in sides:
                    act_msum(mbuf[pref + "3"][:, 1:1 + Rr, 1:C + 1],
                             img * 6 + (0 if pref == "d" else 3))

                # products for m3 while halos fly
                for pref, OP, base_q in sides:
                    pr = prod_pool.tile([P, Rr, C], f16, tag="prod",
                                        name=f"pr3{pref}")
                    nc.vector.tensor_tensor(pr[:],
                                            mbuf[pref + "3"][:, 1:1 + Rr, 1:C + 1],
                                            p[:], op=MULT)
                    pe_sum(base_q + 0, pr[:, :, :])

                # m5 = op(m3 l1, r1, up1, dn1)
                for pref, OP, _ in sides:
                    m3 = mbuf[pref + "3"]
                    m5 = mbuf[pref + "5"]
                    nc.vector.tensor_tensor(m5[:, 1:1 + Rr, 1:C + 1],
                                            m3[:, 1:1 + Rr, 0:C],
                                            m3[:, 1:1 + Rr, 2:C + 2], op=OP)
                    nc.vector.tensor_tensor(m5[:, 1:1 + Rr, 1:C + 1],
                                            m5[:, 1:1 + Rr, 1:C + 1],
                                            m3[:, 0:Rr, 1:C + 1], op=OP)
                    nc.vector.tensor_tensor(m5[:, 1:1 + Rr, 1:C + 1],
                                            m5[:, 1:1 + Rr, 1:C + 1],
                                            m3[:, 2:2 + Rr, 1:C + 1], op=OP)
                for pref, OP, _ in sides:
                    m_halo(mbuf[pref + "5"], pref + "5")
                for pref, OP, _ in sides:
                    act_msum(mbuf[pref + "5"][:, 1:1 + Rr, 1:C + 1],
                             img * 6 + 1 + (0 if pref == "d" else 3))


                # products for m5
                for pref, OP, base_q in sides:
                    pr = prod_pool.tile([P, Rr, C], f16, tag="prod",
                                        name=f"pr5{pref}")
                    nc.vector.tensor_tensor(pr[:],
                                            mbuf[pref + "5"][:, 1:1 + Rr, 1:C + 1],
                                            p[:], op=MULT)
                    pe_sum(base_q + 1, pr[:, :, :])

                # corner terms M = op(t up2, t dn2) into prod-pool tiles
                Ms = {}
                for pref, OP, _ in sides:
                    M = prod_pool.tile([P, Rr, C], f16, tag="prod",
                                       name=f"M{pref}")
                    nc.vector.tensor_tensor(M[:], t[:, 0:Rr, :],
                                            t[:, 4:4 + Rr, :], op=OP)
                    Ms[pref] = M

                # m7 (uses m7t, one side at a time: d fully then e)
                for pref, OP, base_q in sides:
                    m5 = mbuf[pref + "5"]
                    M = Ms[pref]
                    # d-side uses m7t; e-side reuses m3_e's dead interior so
                    # the two sides' buffers are independent (no WAR with the
                    # d-side's ACT sum)
                    m7o = (m7t if pref == "d"
                           else mbuf["e3"][:, 1:1 + Rr, 1:C + 1])
                    nc.vector.tensor_tensor(m7o[:], m5[:, 1:1 + Rr, 0:C],
                                            m5[:, 1:1 + Rr, 2:C + 2], op=OP)
                    nc.vector.tensor_tensor(m7o[:], m7o[:],
                                            m5[:, 0:Rr, 1:C + 1], op=OP)
                    nc.vector.tensor_tensor(m7o[:], m7o[:],
                                            m5[:, 2:2 + Rr, 1:C + 1], op=OP)
                    # corner merges, col-restricted
                    nc.vector.tensor_tensor(m7o[:, :, 2:C], m7o[:, :, 2:C],
                                            M[:, :, 0:C - 2], op=OP)
                    nc.vector.tensor_tensor(m7o[:, :, 0:C - 2], m7o[:, :, 0:C - 2],
                                            M[:, :, 2:C], op=OP)
                    act_msum(m7o[:], img * 6 + 2 + (0 if pref == "d" else 3))
                    # split the m7 product into row pairs so its PE sum chain
                    # overlaps the remaining DVE work (shortens the tail)
                    pr = prod_pool.tile([P, Rr, C], f16, tag="prod",
                                        name=f"pr7{pref}")
                    last = (img == n_img - 1 and pref == "e")
                    chunks = ((0, 2), (2, 4), (4, 6), (6, 7), (7, 8)) if last \
                        else ((0, 2), (2, 4), (4, 6), (6, 8))
                    for ra, rb in chunks:
                        nc.vector.tensor_tensor(pr[:, ra:rb, :],
                                                m7o[:, ra:rb, :],
                                                p[:, ra:rb, :], op=MULT)
                        pe_sum_rows(base_q + 2, pr[:, :, :], ra, rb)

            # ---- epilogue ----
            for q in (0, 1, 2):
                nc.vector.tensor_reduce(outsb[:, q:q + 1], ps_prod[q][:],
                                        axis=mybir.AxisListType.X,
                                        op=mybir.AluOpType.add)
            for q in (3, 4, 5):
                pdump_q(q)
            ps_plain = phalo_pool.tile([1, NPLAIN], f32, tag="phalo",
                                       name="ps_plain")
            nc.tensor.matmul(ps_plain[:], ones32[:], sums[:], start=True,
                             stop=True)
            nc.scalar.activation(outsb[:, NQ:NQ + NPLAIN], ps_plain[:], COPY)
            nc.sync.dma_start(out_dram[:], outsb[:])

    nc.compile()
    return nc


def combine_partials(partials, n_img=BPC):
    """partials: [ncores, NOUT] float32 -> scalar loss."""
    partials = np.asarray(partials, dtype=np.float64)
    prod_sums = partials[:, 0:NQ].sum(axis=0)
    plain = partials[:, NQ:]
    p_sum = plain[:, 0:4].sum()
    m_sums = np.zeros(NQ)
    for img in range(n_img):
        for q in range(NQ):
            slot = img * 6 + q
            m_sums[q] += plain[:, 8 + 2 * slot:10 + 2 * slot].sum()
    total = 0.0
    for q in range(NQ):
        card = p_sum + m_sums[q]
        score = 2.0 * prod_sums[q] / max(card, EPS)
        total += (1.0 - score) * (1.0 if m_sums[q] > 0 else 0.0)
    return np.float32(total / 3.0)


def make_in_maps(pred_student_prob, teacher_prob):
    """Host-side prep: reshape, cast to fp16, shard over cores."""
    pred = np.ascontiguousarray(pred_student_prob.reshape(B, H, W)).astype(np.float16)
    teach = np.ascontiguousarray(teacher_prob.reshape(B, H, W)).astype(np.float16)
    wup = _shift_identity(True)
    wdn = _shift_identity(False)
    in_maps = []
    for c in range(NCORES):
        sl = slice(c * BPC, (c + 1) * BPC)
        in_maps.append({
            "teacher": np.ascontiguousarray(teach[sl]),
            "pred": np.ascontiguousarray(pred[sl]),
            "wup": wup,
            "wdn": wdn,
        })
    return in_maps


def kernel(pred_student_prob, teacher_prob):
    from concourse.bass_utils import run_bass_kernel_spmd

    key = (BPC, R, W)
    if key not in _CACHE:
        _CACHE[key] = build_nc(BPC, R, W)
    nc = _CACHE[key]

    in_maps = make_in_maps(pred_student_prob, teacher_prob)
    res = run_bass_kernel_spmd(nc, in_maps, core_ids=list(range(NCORES)))
    partials = np.stack([res.results[c]["partials"][0] for c in range(NCORES)])
    return combine_partials(partials)


# revision 6
# speedup vs baseline: 1.2771x; 1.0045x over previous
"""Trainium2 Bass kernel for nn_LossConsistenciaMorfologicaCompuesta (v2).

Composite morphological-consistency loss:
  for k in (3,5,7): Dice(pred, dilate_k(teacher)) + Dice(pred, erode_k(teacher)),
  total/3, cv2-style ellipse structuring elements, Dice reduced over
  (batch, pixels).

Strategy (8 NeuronCores, data-parallel over batch B=16 -> 2 images/core):
  - Inputs are cast to fp16 on the host and DMA'd directly into SBUF (no
    on-device casts, half the HBM traffic).
  - Slab layout: image = [128 partitions, 8 rows, 1024 cols] fp16; vertical
    shifts are free-dim row offsets. Slab-crossing halo rows are built by the
    PE with shifted-identity matmuls (partition shift) + ACT PSUM->SBUF
    copies, with replicate edges (exact for flat morphology; partition-127
    edge replicates go via tiny DMAs since engines cannot address them).
  - Ellipse decomposition (verified exact vs reference):
      h1 = hmax3(t); m3 = max(h1, t up1, t dn1)
      m5 = max(m3 l1, r1, up1, dn1)
      m7 = max(m5 l1, r1, up1, dn1) merged with corner term
           M = max(t up2, dn2) shifted +-2 cols
    erosion mirrored with min. 16 full-size DVE passes per (image, side) —
    the minimum for this op set — all at the fp16 2x DVE rate; dil/ero
    emission is interleaved so halo latency hides under compute.
  - Sums: sum(p) and sum(m) via ACT copy into a dump tile with accum_out;
    sum(p*m) via DVE mult into rotating product tiles + PE ones-matmul PSUM
    chains (tapered row chunks so the last chain overlaps the DVE tail).
  - Each core writes 38 partial sums; the host combines them into the loss.
"""

import numpy as np

B, C_IN, H, W = 16, 1, 1024, 1024
NCORES = 8
BPC = B // NCORES      # images per core
P = 128                # SBUF partitions
R = H // P             # 8 slab rows per partition
EPS = 1e-7
CW = 512               # PSUM chunk width

NQ = 6                 # d3,d5,d7,e3,e5,e7
NPLAIN = 32            # plain-sum columns: 8 p-chunks + 24 m-sum halves
NOUT = NQ + NPLAIN

_CACHE = {}


def _shift_identity(up: bool) -> np.ndarray:
    """W[p, k] = 1 iff p == k-1 (up) / p == k+1 (down); edge row zeroed
    (edge halo handled by an explicit replicate copy)."""
    w = np.zeros((P, P), np.float16)
    if up:
        for k in range(1, P):
            w[k - 1, k] = 1.0
    else:
        for k in range(P - 1):
            w[k + 1, k] = 1.0
    return w


def build_nc(n_img=BPC, rows=R, cols=W):
    import concourse.bacc as bacc
    import concourse.mybir as mybir
    import concourse.tile as tile

    f32 = mybir.dt.float32
    f16 = mybir.dt.float16
    MAX = mybir.AluOpType.max
    MIN = mybir.AluOpType.min
    MULT = mybir.AluOpType.mult
    COPY = mybir.ActivationFunctionType.Copy

    Rr, C = rows, cols
    TROWS = Rr + 4          # t: rows idx 0,1 = image rows -2,-1; 2..9 = 0..7;
                            # 10,11 = rows 8,9
    MROWS = Rr + 2          # m3/m5: row idx 0 = halo -1, 1..8 interior, 9 = halo
    MC = C + 2              # 1 pad col each side

    nc = bacc.Bacc("TRN2", target_bir_lowering=False)
    t_dram = nc.dram_tensor("teacher", [n_img, Rr * P, C], f16, kind="ExternalInput")
    p_dram = nc.dram_tensor("pred", [n_img, Rr * P, C], f16, kind="ExternalInput")
    wup_dram = nc.dram_tensor("wup", [P, P], f16, kind="ExternalInput")
    wdn_dram = nc.dram_tensor("wdn", [P, P], f16, kind="ExternalInput")
    out_dram = nc.dram_tensor("partials", [1, NOUT], f32, kind="ExternalOutput")

    with tile.TileContext(nc) as tc:
        with (
            tc.tile_pool(name="img", bufs=1) as img_pool,
            tc.tile_pool(name="morph", bufs=1) as morph_pool,
            tc.tile_pool(name="prod", bufs=3) as prod_pool,
            tc.tile_pool(name="small", bufs=1) as small_pool,
            tc.tile_pool(name="pprod", bufs=1, space="PSUM") as pprod_pool,
            tc.tile_pool(name="phalo", bufs=2, space="PSUM") as phalo_pool,
        ):
            sums = small_pool.tile([P, NPLAIN], f32, tag="sums")
            ones16 = small_pool.tile([P, 1], f16, tag="ones16")
            ones32 = small_pool.tile([P, 1], f32, tag="ones32")
            outsb = small_pool.tile([1, NOUT], f32, tag="outsb")
            wup = small_pool.tile([P, P], f16, tag="wup")
            wdn = small_pool.tile([P, P], f16, tag="wdn")
            nc.vector.memset(sums[:], 0.0)
            nc.vector.memset(ones16[:], 1.0)
            nc.vector.memset(ones32[:], 1.0)


            t = img_pool.tile([P, TROWS, C], f16, tag="t")
            p = img_pool.tile([P, Rr, C], f16, tag="p")
            dump = img_pool.tile([P, Rr // 2, C], f16, tag="dump")
            m7t = img_pool.tile([P, Rr, C], f16, tag="m7t")
            mbuf = {}
            for pref, fill in (("d", -1e4), ("e", 1e4)):
                for lvl in ("3", "5"):
                    m = morph_pool.tile([P, MROWS, MC], f16, tag=pref + lvl,
                                        name=pref + lvl)
                    nc.vector.memset(m[:, :, 0:1], fill)
                    nc.vector.memset(m[:, :, MC - 1:MC], fill)
                    mbuf[pref + lvl] = m

            ps_prod = [pprod_pool.tile([1, CW], f32, tag=f"ps{q}", name=f"ps{q}")
                       for q in range(NQ)]
            nch = (C + CW - 1) // CW
            total_mm = n_img * Rr * nch
            mm_count = [0] * NQ

            def pe_sum_rows(q, m_ap, ra, rb):
                """Accumulate sum over rows [ra, rb) of a [P, Rr, C] AP into
                ps_prod[q] (fp16 ones-matmuls)."""
                for r in range(ra, rb):
                    for c0 in range(0, C, CW):
                        nc.tensor.matmul(
                            ps_prod[q][:, 0:CW],
                            ones16[:],
                            m_ap[:, r, c0:c0 + CW],
                            start=(mm_count[q] == 0),
                            stop=(mm_count[q] == total_mm - 1),
                        )
                        mm_count[q] += 1

            def pe_sum(q, m_ap):
                pe_sum_rows(q, m_ap, 0, Rr)

            def pe_halo_row(dst_row_ap, w_ap, src_row_ap, tag):
                """dst_row[1:P or 0:P-1] = partition-shifted src_row via PE;
                edge partition left for a separate replicate copy."""
                for ci, c0 in enumerate(range(0, C, CW)):
                    ph = phalo_pool.tile([P, CW], f32, tag="phalo",
                                         name=f"ph_{tag}_{ci}")
                    nc.tensor.matmul(ph[:], w_ap[:], src_row_ap[:, c0:c0 + CW],
                                     start=True, stop=True)
                    nc.scalar.activation(dst_row_ap[:, c0:c0 + CW], ph[:, :],
                                         COPY)

            def m_halo(m, side):
                """Fill m's halo rows 0 and MROWS-1 (interior cols) via PE;
                replicate at image top/bottom edges."""
                pe_halo_row(m[:, 0, 1:C + 1], wup, m[:, MROWS - 2, 1:C + 1],
                            f"{side}u")
                pe_halo_row(m[:, MROWS - 1, 1:C + 1], wdn, m[:, 1, 1:C + 1],
                            f"{side}d")
                nc.scalar.activation(m[0:1, 0, 1:C + 1], m[0:1, 1, 1:C + 1], COPY)
                nc.sync.dma_start(m[P - 1:P, MROWS - 1:MROWS, 1:C + 1],
                                  m[P - 1:P, MROWS - 2:MROWS - 1, 1:C + 1])

            def act_msum(m_ap, slot):
                """sum(m) on ACT in two half-dumps; accum cols 8+2*slot, +1."""
                half = Rr // 2
                for hi in range(2):
                    nc.scalar.activation(dump[:], m_ap[:, hi * half:(hi + 1) * half, :],
                                         COPY,
                                         accum_out=sums[:, 8 + 2 * slot + hi:
                                                        9 + 2 * slot + hi])

            pdump = small_pool.tile([1, CW], f32, tag="pdump")
            pdumped = set()

            def pdump_q(q):
                pdumped.add(q)
                nc.scalar.activation(pdump[:], ps_prod[q][:], COPY,
                                     accum_out=outsb[:, q:q + 1])

            for img in range(n_img):
                t_view = t_dram[img].rearrange("(p r) w -> p r w", p=P)
                p_view = p_dram[img].rearrange("(p r) w -> p r w", p=P)
                CH = 2
                # ---- teacher load: direct fp16 DMA (host pre-casts).
                # img 0 uses small leading chunks so h1 starts early. ----
                tchunks = ((0, 1), (1, 2), (2, 4), (4, 8)) if img == 0 \
                    else ((0, 4), (4, 8))
                for r0, r1 in tchunks:
                    nc.sync.dma_start(t[:, 2 + r0:2 + r1, :],
                                      t_view[:, r0:r1, :])
                if img == 0:
                    nc.sync.dma_start(wup[:], wup_dram[:])
                    nc.sync.dma_start(wdn[:], wdn_dram[:])
                # ---- t halo rows via PE shift + replicate edges ----
                pe_halo_row(t[:, 1, :], wup, t[:, 9, :], "t1")   # row -1
                pe_halo_row(t[:, 0, :], wup, t[:, 8, :], "t0")   # row -2
                pe_halo_row(t[:, 10, :], wdn, t[:, 2, :], "t10")  # row 8
                pe_halo_row(t[:, 11, :], wdn, t[:, 3, :], "t11")  # row 9
                for hr in (0, 1):
                    nc.scalar.activation(t[0:1, hr, :], t[0:1, 2, :], COPY)
                for hr in (10, 11):
                    nc.sync.dma_start(t[P - 1:P, hr:hr + 1, :],
                                      t[P - 1:P, 9:10, :])
                # ---- pred load: direct fp16 DMA; sum(p) via ACT dumps ----
                for r0, r1 in ((0, 4), (4, 8)):
                    nc.sync.dma_start(p[:, r0:r1, :], p_view[:, r0:r1, :])
                half = Rr // 2
                for hi in range(2):
                    nc.scalar.activation(dump[:], p[:, hi * half:(hi + 1) * half, :],
                                         COPY,
                                         accum_out=sums[:, img * 2 + hi:
                                                        img * 2 + hi + 1])

                sides = (("d", MAX, 0), ("e", MIN, 3))

                # h1 into m5 interior (alias; dead once m5 is written).
                # For img 0 split into row halves so work starts after the
                # first two cast chunks.
                h1 = {pref: mbuf[pref + "5"][:, 1:1 + Rr, 1:C + 1]
                      for pref, _, _ in sides}
                ti = t[:, 2:2 + Rr, :]     # image rows 0..7
                row_chunks = ((0, 1), (1, 2), (2, 4), (4, 8)) if img == 0 else ((0, 8),)
                for ra, rb in row_chunks:
                    for pref, OP, _ in sides:
                        h = h1[pref][:, ra:rb, :]
                        tc_ = ti[:, ra:rb, :]
                        nc.vector.tensor_tensor(h[:, :, 1:C - 1], tc_[:, :, 0:C - 2],
                                                tc_[:, :, 2:C], op=OP)
                        nc.vector.tensor_tensor(h[:, :, 1:C - 1], h[:, :, 1:C - 1],
                                                tc_[:, :, 1:C - 1], op=OP)
                        nc.vector.tensor_tensor(h[:, :, 0:1], tc_[:, :, 0:1],
                                                tc_[:, :, 1:2], op=OP)
                        nc.vector.tensor_tensor(h[:, :, C - 1:C],
                                                tc_[:, :, C - 2:C - 1],
                                                tc_[:, :, C - 1:C], op=OP)

                # m3 = op(h1, t up1, t dn1)
                for pref, OP, _ in sides:
                    m3 = mbuf[pref + "3"]
                    nc.vector.tensor_tensor(m3[:, 1:1 + Rr, 1:C + 1], h1[pref],
                                            t[:, 1:1 + Rr, :], op=OP)
                    nc.vector.tensor_tensor(m3[:, 1:1 + Rr, 1:C + 1],
                                            m3[:, 1:1 + Rr, 1:C + 1],
                                            t[:, 3:3 + Rr, :], op=OP)
                for pref, OP, _ in sides:
                    m_halo(mbuf[pref + "3"], pref + "3")
                for pref, OP, _ in sides:
                    act_msum(mbuf[pref + "3"][:, 1:1 + Rr, 1:C + 1],
                             img * 6 + (0 if pref == "d" else 3))

                # products for m3 while halos fly
                for pref, OP, base_q in sides:
                    pr = prod_pool.tile([P, Rr, C], f16, tag="prod",
                                        name=f"pr3{pref}")
                    nc.vector.tensor_tensor(pr[:],
                                            mbuf[pref + "3"][:, 1:1 + Rr, 1:C + 1],
                                            p[:], op=MULT)
                    pe_sum(base_q + 0, pr[:, :, :])

                # m5 = op(m3 l1, r1, up1, dn1)
                for pref, OP, _ in sides:
                    m3 = mbuf[pref + "3"]
                    m5 = mbuf[pref + "5"]
                    nc.vector.tensor_tensor(m5[:, 1:1 + Rr, 1:C + 1],
                                            m3[:, 1:1 + Rr, 0:C],
                                            m3[:, 1:1 + Rr, 2:C + 2], op=OP)
                    nc.vector.tensor_tensor(m5[:, 1:1 + Rr, 1:C + 1],
                                            m5[:, 1:1 + Rr, 1:C + 1],
                                            m3[:, 0:Rr, 1:C + 1], op=OP)
                    nc.vector.tensor_tensor(m5[:, 1:1 + Rr, 1:C + 1],
                                            m5[:, 1:1 + Rr, 1:C + 1],
                                            m3[:, 2:2 + Rr, 1:C + 1], op=OP)
                for pref, OP, _ in sides:
                    m_halo(mbuf[pref + "5"], pref + "5")
                for pref, OP, _ in sides:
                    act_msum(mbuf[pref + "5"][:, 1:1 + Rr, 1:C + 1],
                             img * 6 + 1 + (0 if pref == "d" else 3))


                # products for m5
                for pref, OP, base_q in sides:
                    pr = prod_pool.tile([P, Rr, C], f16, tag="prod",
                                        name=f"pr5{pref}")
                    nc.vector.tensor_tensor(pr[:],
                                            mbuf[pref + "5"][:, 1:1 + Rr, 1:C + 1],
                                            p[:], op=MULT)
                    pe_sum(base_q + 1, pr[:, :, :])

                # corner terms M = op(t up2, t dn2) into prod-pool tiles
                Ms = {}
                for pref, OP, _ in sides:
                    M = prod_pool.tile([P, Rr, C], f16, tag="prod",
                                       name=f"M{pref}")
                    nc.vector.tensor_tensor(M[:], t[:, 0:Rr, :],
                                            t[:, 4:4 + Rr, :], op=OP)
                    Ms[pref] = M

                # m7 (uses m7t, one side at a time: d fully then e)
                for pref, OP, base_q in sides:
                    m5 = mbuf[pref + "5"]
                    M = Ms[pref]
                    # d-side uses m7t; e-side reuses m3_e's dead interior so
                    # the two sides' buffers are independent (no WAR with the
                    # d-side's ACT sum)
                    m7o = (m7t if pref == "d"
                           else mbuf["e3"][:, 1:1 + Rr, 1:C + 1])
                    nc.vector.tensor_tensor(m7o[:], m5[:, 1:1 + Rr, 0:C],
                                            m5[:, 1:1 + Rr, 2:C + 2], op=OP)
                    nc.vector.tensor_tensor(m7o[:], m7o[:],
                                            m5[:, 0:Rr, 1:C + 1], op=OP)
                    nc.vector.tensor_tensor(m7o[:], m7o[:],
                                            m5[:, 2:2 + Rr, 1:C + 1], op=OP)
                    # corner merges, col-restricted
                    nc.vector.tensor_tensor(m7o[:, :, 2:C], m7o[:, :, 2:C],
                                            M[:, :, 0:C - 2], op=OP)
                    nc.vector.tensor_tensor(m7o[:, :, 0:C - 2], m7o[:, :, 0:C - 2],
                                            M[:, :, 2:C], op=OP)
                    act_msum(m7o[:], img * 6 + 2 + (0 if pref == "d" else 3))
                    # split the m7 product into row pairs so its PE sum chain
                    # overlaps the remaining DVE work (shortens the tail)
                    pr = prod_pool.tile([P, Rr, C], f16, tag="prod",
                                        name=f"pr7{pref}")
                    last = (img == n_img - 1 and pref == "e")
                    chunks = ((0, 2), (2, 4), (4, 6), (6, 7), (7, 8)) if last \
                        else ((0, 2), (2, 4), (4, 6), (6, 8))
                    for ra, rb in chunks:
                        nc.vector.tensor_tensor(pr[:, ra:rb, :],
                                                m7o[:, ra:rb, :],
                                                p[:, ra:rb, :], op=MULT)
                        pe_sum_rows(base_q + 2, pr[:, :, :], ra, rb)

            # ---- epilogue ----
            for q in (0, 1, 2):
                nc.vector.tensor_reduce(outsb[:, q:q + 1], ps_prod[q][:],
                                        axis=mybir.AxisListType.X,
                                        op=mybir.AluOpType.add)
            for q in (3, 4, 5):
                pdump_q(q)
            ps_plain = phalo_pool.tile([1, NPLAIN], f32, tag="phalo",
                                       name="ps_plain")
            nc.tensor.matmul(ps_plain[:], ones32[:], sums[:], start=True,
                             stop=True)
            nc.scalar.activation(outsb[:, NQ:NQ + NPLAIN], ps_plain[:], COPY)
            nc.sync.dma_start(out_dram[:], outsb[:])

    nc.compile()
    return nc


def combine_partials(partials, n_img=BPC):
    """partials: [ncores, NOUT] float32 -> scalar loss."""
    partials = np.asarray(partials, dtype=np.float64)
    prod_sums = partials[:, 0:NQ].sum(axis=0)
    plain = partials[:, NQ:]
    p_sum = plain[:, 0:4].sum()
    m_sums = np.zeros(NQ)
    for img in range(n_img):
        for q in range(NQ):
            slot = img * 6 + q
            m_sums[q] += plain[:, 8 + 2 * slot:10 + 2 * slot].sum()
    total = 0.0
    for q in range(NQ):
        card = p_sum + m_sums[q]
        score = 2.0 * prod_sums[q] / max(card, EPS)
        total += (1.0 - score) * (1.0 if m_sums[q] > 0 else 0.0)
    return np.float32(total / 3.0)


def make_in_maps(pred_student_prob, teacher_prob):
    """Host-side prep: reshape, cast to fp16, shard over cores."""
    pred = np.ascontiguousarray(pred_student_prob.reshape(B, H, W)).astype(np.float16)
    teach = np.ascontiguousarray(teacher_prob.reshape(B, H, W)).astype(np.float16)
    wup = _shift_identity(True)
    wdn = _shift_identity(False)
    in_maps = []
    for c in range(NCORES):
        sl = slice(c * BPC, (c + 1) * BPC)
        in_maps.append({
            "teacher": np.ascontiguousarray(teach[sl]),
            "pred": np.ascontiguousarray(pred[sl]),
            "wup": wup,
            "wdn": wdn,
        })
    return in_maps


def kernel(pred_student_prob, teacher_prob):
    from concourse.bass_utils import run_bass_kernel_spmd

    key = (BPC, R, W)
    if key not in _CACHE:
        _CACHE[key] = build_nc(BPC, R, W)
    nc = _CACHE[key]

    in_maps = make_in_maps(pred_student_prob, teacher_prob)
    res = run_bass_kernel_spmd(nc, in_maps, core_ids=list(range(NCORES)))
    partials = np.stack([res.results[c]["partials"][0] for c in range(NCORES)])
    return combine_partials(partials)


# revision 7
# speedup vs baseline: 1.2777x; 1.0004x over previous
"""Trainium2 Bass kernel for nn_LossConsistenciaMorfologicaCompuesta (v2).

Composite morphological-consistency loss:
  for k in (3,5,7): Dice(pred, dilate_k(teacher)) + Dice(pred, erode_k(teacher)),
  total/3, cv2-style ellipse structuring elements, Dice reduced over
  (batch, pixels).

Strategy (8 NeuronCores, data-parallel over batch B=16 -> 2 images/core):
  - Inputs are cast to fp16 on the host and DMA'd directly into SBUF (no
    on-device casts, half the HBM traffic).
  - Slab layout: image = [128 partitions, 8 rows, 1024 cols] fp16; vertical
    shifts are free-dim row offsets. Slab-crossing halo rows are built by the
    PE with shifted-identity matmuls (partition shift) + ACT PSUM->SBUF
    copies, with replicate edges (exact for flat morphology; partition-127
    edge replicates go via tiny DMAs since engines cannot address them).
  - Ellipse decomposition (verified exact vs reference):
      h1 = hmax3(t); m3 = max(h1, t up1, t dn1)
      m5 = max(m3 l1, r1, up1, dn1)
      m7 = max(m5 l1, r1, up1, dn1) merged with corner term
           M = max(t up2, dn2) shifted +-2 cols
    erosion mirrored with min. 16 full-size DVE passes per (image, side) —
    the minimum for this op set — all at the fp16 2x DVE rate; dil/ero
    emission is interleaved so halo latency hides under compute.
  - Sums: sum(p) and sum(m) via ACT copy into a dump tile with accum_out;
    sum(p*m) via DVE mult into rotating product tiles + PE ones-matmul PSUM
    chains (tapered row chunks so the last chain overlaps the DVE tail).
  - Each core writes 38 partial sums; the host combines them into the loss.
"""

import numpy as np

B, C_IN, H, W = 16, 1, 1024, 1024
NCORES = 8
BPC = B // NCORES      # images per core
P = 128                # SBUF partitions
R = H // P             # 8 slab rows per partition
EPS = 1e-7
CW = 512               # PSUM chunk width

NQ = 6                 # d3,d5,d7,e3,e5,e7
NPLAIN = 32            # plain-sum columns: 8 p-chunks + 24 m-sum halves
NOUT = NQ + NPLAIN

_CACHE = {}


def _shift_identity(up: bool) -> np.ndarray:
    """W[p, k] = 1 iff p == k-1 (up) / p == k+1 (down); edge row zeroed
    (edge halo handled by an explicit replicate copy)."""
    w = np.zeros((P, P), np.float16)
    if up:
        for k in range(1, P):
            w[k - 1, k] = 1.0
    else:
        for k in range(P - 1):
            w[k + 1, k] = 1.0
    return w


def build_nc(n_img=BPC, rows=R, cols=W):
    import concourse.bacc as bacc
    import concourse.mybir as mybir
    import concourse.tile as tile

    f32 = mybir.dt.float32
    f16 = mybir.dt.float16
    MAX = mybir.AluOpType.max
    MIN = mybir.AluOpType.min
    MULT = mybir.AluOpType.mult
    COPY = mybir.ActivationFunctionType.Copy

    Rr, C = rows, cols
    TROWS = Rr + 4          # t: rows idx 0,1 = image rows -2,-1; 2..9 = 0..7;
                            # 10,11 = rows 8,9
    MROWS = Rr + 2          # m3/m5: row idx 0 = halo -1, 1..8 interior, 9 = halo
    MC = C + 2              # 1 pad col each side

    nc = bacc.Bacc("TRN2", target_bir_lowering=False)
    t_dram = nc.dram_tensor("teacher", [n_img, Rr * P, C], f16, kind="ExternalInput")
    p_dram = nc.dram_tensor("pred", [n_img, Rr * P, C], f16, kind="ExternalInput")
    wup_dram = nc.dram_tensor("wup", [P, P], f16, kind="ExternalInput")
    wdn_dram = nc.dram_tensor("wdn", [P, P], f16, kind="ExternalInput")
    out_dram = nc.dram_tensor("partials", [1, NOUT], f32, kind="ExternalOutput")

    with tile.TileContext(nc) as tc:
        with (
            tc.tile_pool(name="img", bufs=1) as img_pool,
            tc.tile_pool(name="morph", bufs=1) as morph_pool,
            tc.tile_pool(name="prod", bufs=3) as prod_pool,
            tc.tile_pool(name="small", bufs=1) as small_pool,
            tc.tile_pool(name="pprod", bufs=1, space="PSUM") as pprod_pool,
            tc.tile_pool(name="phalo", bufs=2, space="PSUM") as phalo_pool,
        ):
            sums = small_pool.tile([P, NPLAIN], f32, tag="sums")
            ones16 = small_pool.tile([P, 1], f16, tag="ones16")
            ones32 = small_pool.tile([P, 1], f32, tag="ones32")
            outsb = small_pool.tile([1, NOUT], f32, tag="outsb")
            wup = small_pool.tile([P, P], f16, tag="wup")
            wdn = small_pool.tile([P, P], f16, tag="wdn")
            nc.vector.memset(sums[:], 0.0)
            nc.vector.memset(ones16[:], 1.0)
            nc.vector.memset(ones32[:], 1.0)


            t = img_pool.tile([P, TROWS, C], f16, tag="t")
            p = img_pool.tile([P, Rr, C], f16, tag="p")
            dump = img_pool.tile([P, Rr // 2, C], f16, tag="dump")
            m7t = img_pool.tile([P, Rr, C], f16, tag="m7t")
            mbuf = {}
            for pref, fill in (("d", -1e4), ("e", 1e4)):
                for lvl in ("3", "5"):
                    m = morph_pool.tile([P, MROWS, MC], f16, tag=pref + lvl,
                                        name=pref + lvl)
                    nc.vector.memset(m[:, :, 0:1], fill)
                    nc.vector.memset(m[:, :, MC - 1:MC], fill)
                    mbuf[pref + lvl] = m

            ps_prod = [pprod_pool.tile([1, CW], f32, tag=f"ps{q}", name=f"ps{q}")
                       for q in range(NQ)]
            nch = (C + CW - 1) // CW
            total_mm = n_img * Rr * nch
            mm_count = [0] * NQ

            def pe_sum_rows(q, m_ap, ra, rb):
                """Accumulate sum over rows [ra, rb) of a [P, Rr, C] AP into
                ps_prod[q] (fp16 ones-matmuls)."""
                for r in range(ra, rb):
                    for c0 in range(0, C, CW):
                        nc.tensor.matmul(
                            ps_prod[q][:, 0:CW],
                            ones16[:],
                            m_ap[:, r, c0:c0 + CW],
                            start=(mm_count[q] == 0),
                            stop=(mm_count[q] == total_mm - 1),
                        )
                        mm_count[q] += 1

            def pe_sum(q, m_ap):
                pe_sum_rows(q, m_ap, 0, Rr)

            def pe_halo_row(dst_row_ap, w_ap, src_row_ap, tag):
                """dst_row[1:P or 0:P-1] = partition-shifted src_row via PE;
                edge partition left for a separate replicate copy."""
                for ci, c0 in enumerate(range(0, C, CW)):
                    ph = phalo_pool.tile([P, CW], f32, tag="phalo",
                                         name=f"ph_{tag}_{ci}")
                    nc.tensor.matmul(ph[:], w_ap[:], src_row_ap[:, c0:c0 + CW],
                                     start=True, stop=True)
                    nc.scalar.activation(dst_row_ap[:, c0:c0 + CW], ph[:, :],
                                         COPY)

            def m_halo(m, side):
                """Fill m's halo rows 0 and MROWS-1 (interior cols) via PE;
                replicate at image top/bottom edges."""
                pe_halo_row(m[:, 0, 1:C + 1], wup, m[:, MROWS - 2, 1:C + 1],
                            f"{side}u")
                pe_halo_row(m[:, MROWS - 1, 1:C + 1], wdn, m[:, 1, 1:C + 1],
                            f"{side}d")
                nc.scalar.activation(m[0:1, 0, 1:C + 1], m[0:1, 1, 1:C + 1], COPY)
                nc.sync.dma_start(m[P - 1:P, MROWS - 1:MROWS, 1:C + 1],
                                  m[P - 1:P, MROWS - 2:MROWS - 1, 1:C + 1])

            def act_msum(m_ap, slot):
                """sum(m) on ACT in two half-dumps; accum cols 8+2*slot, +1."""
                half = Rr // 2
                for hi in range(2):
                    nc.scalar.activation(dump[:], m_ap[:, hi * half:(hi + 1) * half, :],
                                         COPY,
                                         accum_out=sums[:, 8 + 2 * slot + hi:
                                                        9 + 2 * slot + hi])

            pdump = small_pool.tile([1, CW], f32, tag="pdump")
            pdumped = set()

            def pdump_q(q):
                pdumped.add(q)
                nc.scalar.activation(pdump[:], ps_prod[q][:], COPY,
                                     accum_out=outsb[:, q:q + 1])

            for img in range(n_img):
                t_view = t_dram[img].rearrange("(p r) w -> p r w", p=P)
                p_view = p_dram[img].rearrange("(p r) w -> p r w", p=P)
                CH = 2
                # ---- teacher load: direct fp16 DMA (host pre-casts).
                # img 0 uses small leading chunks so h1 starts early. ----
                tchunks = ((0, 1), (1, 2), (2, 4), (4, 8)) if img == 0 \
                    else ((0, 4), (4, 8))
                for r0, r1 in tchunks:
                    nc.sync.dma_start(t[:, 2 + r0:2 + r1, :],
                                      t_view[:, r0:r1, :])
                if img == 0:
                    nc.sync.dma_start(wup[:], wup_dram[:])
                    nc.sync.dma_start(wdn[:], wdn_dram[:])
                # ---- t halo rows via PE shift + replicate edges ----
                pe_halo_row(t[:, 1, :], wup, t[:, 9, :], "t1")   # row -1
                pe_halo_row(t[:, 0, :], wup, t[:, 8, :], "t0")   # row -2
                pe_halo_row(t[:, 10, :], wdn, t[:, 2, :], "t10")  # row 8
                pe_halo_row(t[:, 11, :], wdn, t[:, 3, :], "t11")  # row 9
                for hr in (0, 1):
                    nc.scalar.activation(t[0:1, hr, :], t[0:1, 2, :], COPY)
                for hr in (10, 11):
                    nc.sync.dma_start(t[P - 1:P, hr:hr + 1, :],
                                      t[P - 1:P, 9:10, :])
                # ---- pred load: direct fp16 DMA; sum(p) via ACT dumps ----
                for r0, r1 in ((0, 4), (4, 8)):
                    nc.sync.dma_start(p[:, r0:r1, :], p_view[:, r0:r1, :])
                half = Rr // 2
                for hi in range(2):
                    nc.scalar.activation(dump[:], p[:, hi * half:(hi + 1) * half, :],
                                         COPY,
                                         accum_out=sums[:, img * 2 + hi:
                                                        img * 2 + hi + 1])

                sides = (("d", MAX, 0), ("e", MIN, 3))

                # h1 into m5 interior (alias; dead once m5 is written).
                # For img 0 split into row halves so work starts after the
                # first two cast chunks.
                h1 = {pref: mbuf[pref + "5"][:, 1:1 + Rr, 1:C + 1]
                      for pref, _, _ in sides}
                ti = t[:, 2:2 + Rr, :]     # image rows 0..7
                row_chunks = ((0, 1), (1, 2), (2, 4), (4, 8)) if img == 0 else ((0, 8),)
                for ra, rb in row_chunks:
                    for pref, OP, _ in sides:
                        h = h1[pref][:, ra:rb, :]
                        tc_ = ti[:, ra:rb, :]
                        nc.vector.tensor_tensor(h[:, :, 1:C - 1], tc_[:, :, 0:C - 2],
                                                tc_[:, :, 2:C], op=OP)
                        nc.vector.tensor_tensor(h[:, :, 1:C - 1], h[:, :, 1:C - 1],
                                                tc_[:, :, 1:C - 1], op=OP)
                # edge columns once per side over all rows
                for pref, OP, _ in sides:
                    h = h1[pref]
                    nc.vector.tensor_tensor(h[:, :, 0:1], ti[:, :, 0:1],
                                            ti[:, :, 1:2], op=OP)
                    nc.vector.tensor_tensor(h[:, :, C - 1:C],
                                            ti[:, :, C - 2:C - 1],
                                            ti[:, :, C - 1:C], op=OP)

                # m3 = op(h1, t up1, t dn1)
                for pref, OP, _ in sides:
                    m3 = mbuf[pref + "3"]
                    nc.vector.tensor_tensor(m3[:, 1:1 + Rr, 1:C + 1], h1[pref],
                                            t[:, 1:1 + Rr, :], op=OP)
                    nc.vector.tensor_tensor(m3[:, 1:1 + Rr, 1:C + 1],
                                            m3[:, 1:1 + Rr, 1:C + 1],
                                            t[:, 3:3 + Rr, :], op=OP)
                for pref, OP, _ in sides:
                    m_halo(mbuf[pref + "3"], pref + "3")
                for pref, OP, _ in sides:
                    act_msum(mbuf[pref + "3"][:, 1:1 + Rr, 1:C + 1],
                             img * 6 + (0 if pref == "d" else 3))

                # products for m3 while halos fly
                for pref, OP, base_q in sides:
                    pr = prod_pool.tile([P, Rr, C], f16, tag="prod",
                                        name=f"pr3{pref}")
                    nc.vector.tensor_tensor(pr[:],
                                            mbuf[pref + "3"][:, 1:1 + Rr, 1:C + 1],
                                            p[:], op=MULT)
                    pe_sum(base_q + 0, pr[:, :, :])

                # m5 = op(m3 l1, r1, up1, dn1)
                for pref, OP, _ in sides:
                    m3 = mbuf[pref + "3"]
                    m5 = mbuf[pref + "5"]
                    nc.vector.tensor_tensor(m5[:, 1:1 + Rr, 1:C + 1],
                                            m3[:, 1:1 + Rr, 0:C],
                                            m3[:, 1:1 + Rr, 2:C + 2], op=OP)
                    nc.vector.tensor_tensor(m5[:, 1:1 + Rr, 1:C + 1],
                                            m5[:, 1:1 + Rr, 1:C + 1],
                                            m3[:, 0:Rr, 1:C + 1], op=OP)
                    nc.vector.tensor_tensor(m5[:, 1:1 + Rr, 1:C + 1],
                                            m5[:, 1:1 + Rr, 1:C + 1],
                                            m3[:, 2:2 + Rr, 1:C + 1], op=OP)
                for pref, OP, _ in sides:
                    m_halo(mbuf[pref + "5"], pref + "5")
                for pref, OP, _ in sides:
                    act_msum(mbuf[pref + "5"][:, 1:1 + Rr, 1:C + 1],
                             img * 6 + 1 + (0 if pref == "d" else 3))


                # products for m5
                for pref, OP, base_q in sides:
                    pr = prod_pool.tile([P, Rr, C], f16, tag="prod",
                                        name=f"pr5{pref}")
                    nc.vector.tensor_tensor(pr[:],
                                            mbuf[pref + "5"][:, 1:1 + Rr, 1:C + 1],
                                            p[:], op=MULT)
                    pe_sum(base_q + 1, pr[:, :, :])

                # corner terms M = op(t up2, t dn2) into prod-pool tiles
                Ms = {}
                for pref, OP, _ in sides:
                    M = prod_pool.tile([P, Rr, C], f16, tag="prod",
                                       name=f"M{pref}")
                    nc.vector.tensor_tensor(M[:], t[:, 0:Rr, :],
                                            t[:, 4:4 + Rr, :], op=OP)
                    Ms[pref] = M

                # m7 (uses m7t, one side at a time: d fully then e)
                for pref, OP, base_q in sides:
                    m5 = mbuf[pref + "5"]
                    M = Ms[pref]
                    # d-side uses m7t; e-side reuses m3_e's dead interior so
                    # the two sides' buffers are independent (no WAR with the
                    # d-side's ACT sum)
                    m7o = (m7t if pref == "d"
                           else mbuf["e3"][:, 1:1 + Rr, 1:C + 1])
                    nc.vector.tensor_tensor(m7o[:], m5[:, 1:1 + Rr, 0:C],
                                            m5[:, 1:1 + Rr, 2:C + 2], op=OP)
                    nc.vector.tensor_tensor(m7o[:], m7o[:],
                                            m5[:, 0:Rr, 1:C + 1], op=OP)
                    nc.vector.tensor_tensor(m7o[:], m7o[:],
                                            m5[:, 2:2 + Rr, 1:C + 1], op=OP)
                    # corner merges, col-restricted
                    nc.vector.tensor_tensor(m7o[:, :, 2:C], m7o[:, :, 2:C],
                                            M[:, :, 0:C - 2], op=OP)
                    nc.vector.tensor_tensor(m7o[:, :, 0:C - 2], m7o[:, :, 0:C - 2],
                                            M[:, :, 2:C], op=OP)
                    act_msum(m7o[:], img * 6 + 2 + (0 if pref == "d" else 3))
                    # split the m7 product into row pairs so its PE sum chain
                    # overlaps the remaining DVE work (shortens the tail)
                    pr = prod_pool.tile([P, Rr, C], f16, tag="prod",
                                        name=f"pr7{pref}")
                    last = (img == n_img - 1 and pref == "e")
                    chunks = ((0, 2), (2, 4), (4, 6), (6, 7), (7, 8)) if last \
                        else ((0, 2), (2, 4), (4, 6), (6, 8))
                    for ra, rb in chunks:
                        nc.vector.tensor_tensor(pr[:, ra:rb, :],
                                                m7o[:, ra:rb, :],
                                                p[:, ra:rb, :], op=MULT)
                        pe_sum_rows(base_q + 2, pr[:, :, :], ra, rb)

            # ---- epilogue ----
            for q in (0, 1, 2):
                nc.vector.tensor_reduce(outsb[:, q:q + 1], ps_prod[q][:],
                                        axis=mybir.AxisListType.X,
                                        op=mybir.AluOpType.add)
            for q in (3, 4, 5):
                pdump_q(q)
            ps_plain = phalo_pool.tile([1, NPLAIN], f32, tag="phalo",
                                       name="ps_plain")
            nc.tensor.matmul(ps_plain[:], ones32[:], sums[:], start=True,
                             stop=True)
            nc.scalar.activation(outsb[:, NQ:NQ + NPLAIN], ps_plain[:], COPY)
            nc.sync.dma_start(out_dram[:], outsb[:])

    nc.compile()
    return nc


def combine_partials(partials, n_img=BPC):
    """partials: [ncores, NOUT] float32 -> scalar loss."""
    partials = np.asarray(partials, dtype=np.float64)
    prod_sums = partials[:, 0:NQ].sum(axis=0)
    plain = partials[:, NQ:]
    p_sum = plain[:, 0:4].sum()
    m_sums = np.zeros(NQ)
    for img in range(n_img):
        for q in range(NQ):
            slot = img * 6 + q
            m_sums[q] += plain[:, 8 + 2 * slot:10 + 2 * slot].sum()
    total = 0.0
    for q in range(NQ):
        card = p_sum + m_sums[q]
        score = 2.0 * prod_sums[q] / max(card, EPS)
        total += (1.0 - score) * (1.0 if m_sums[q] > 0 else 0.0)
    return np.float32(total / 3.0)


def make_in_maps(pred_student_prob, teacher_prob):
    """Host-side prep: reshape, cast to fp16, shard over cores."""
    pred = np.ascontiguousarray(pred_student_prob.reshape(B, H, W)).astype(np.float16)
    teach = np.ascontiguousarray(teacher_prob.reshape(B, H, W)).astype(np.float16)
    wup = _shift_identity(True)
    wdn = _shift_identity(False)
    in_maps = []
    for c in range(NCORES):
        sl = slice(c * BPC, (c + 1) * BPC)
        in_maps.append({
            "teacher": np.ascontiguousarray(teach[sl]),
            "pred": np.ascontiguousarray(pred[sl]),
            "wup": wup,
            "wdn": wdn,
        })
    return in_maps


def kernel(pred_student_prob, teacher_prob):
    from concourse.bass_utils import run_bass_kernel_spmd

    key = (BPC, R, W)
    if key not in _CACHE:
        _CACHE[key] = build_nc(BPC, R, W)
    nc = _CACHE[key]

    in_maps = make_in_maps(pred_student_prob, teacher_prob)
    res = run_bass_kernel_spmd(nc, in_maps, core_ids=list(range(NCORES)))
    partials = np.stack([res.results[c]["partials"][0] for c in range(NCORES)])
    return combine_partials(partials)
